# revision 24
# baseline (speedup 1.0000x reference)
"""Trainium2 Bass kernel for nn_Block_27187142983954 (dense transformer block,
per-position head-mixing attention). Data-parallel over batch: 8 cores, one
batch element each. Self-contained: hardcodes all shapes.

v2 design (per 128-position tile, S=4096, E=1024, H=16, D=64 per core):
  - qkv projection on TensorE, e-outer loop so each stationary (a 128x128
    feature-major x chunk) is loaded once per psum wave (16 LDW/tile).
  - q.k scores on VectorE: broadcast bf16 product + full halving-tree
    reduction, scoresT layout [pos, g, h] so softmax attn lands transposed.
  - softmax without max-subtraction; attn pre-normalized by 1/den.
  - attn@v on TensorE via block-diagonal stationaries: 16 matmuls of
    [128x128]x[128,64], stationary = 8-position block-diag of 16x16 attn^T
    blocks, moving = v regrouped [(pos8,g), d]. Layout changes ride
    SBUF->DRAM->SBUF DMA (access patterns can't cross partitions on-chip).
  - feature-major transposes for proj/ff inputs via DMA xbar transpose
    (dma_start transpose=True) instead of PE transposes.
  - LayerNorm stats on ScalarE via activation accum_out (Identity/Square);
    rsigma = exp(-0.5*ln(var+eps)); LN1 g/b folded into ff weights on host;
    LN2 g/b applied on host after the kernel.
"""

import sys

sys.path.insert(0, "/opt/trn_rl_repo")

import numpy as np
import ml_dtypes

E, H, DQ, DV = 1024, 16, 64, 64
B, S = 8, 4096
EPS = 1e-5
NT = S // 128  # 32 position tiles per core
BF = ml_dtypes.bfloat16

_CACHE = {}


def _patch_tail_drain():
    """walrus in this container rejects >1 sem wait on a CTRL (Drain)
    instruction; spread the TileContext tail-drain waits over wait-nops."""
    import concourse.tile as tile
    import bass_rust
    from concourse.vector_clock import ScopedClock

    if getattr(tile.TileContext, "_drain_patched", False):
        return

    def _drain_and_barrier(self, tick_clock, wait_clock):
        nc = self.nc
        drain_inst = nc.sync.drain()
        wait_clock.add_sem_waits(
            drain_inst.ins, ScopedClock({None: tick_clock.global_clock})
        )
        si = drain_inst.ins.sync_info
        waits = list(si.on_wait) if si is not None else []
        if len(waits) > 1:
            drain_inst.ins.sync_info = bass_rust.SyncInfo(on_wait=[], on_update=[])
            for w in waits:
                nop = nc.sync.nop()
                nop.ins.sync_info = bass_rust.SyncInfo(on_wait=[w], on_update=[])
        nc.all_engine_barrier()
        assert self.sems is not None
        popped = nc._tile_sem_poison_stack.pop()
        assert popped is self._sem_poison
        nc.clear_and_free_semaphores(list(self.sems.allocated().values()))
        nc.all_engine_barrier()

    tile.TileContext._drain_and_barrier = _drain_and_barrier
    tile.TileContext._drain_patched = True


def _split_excess_waits(nc, max_on_op=1, max_on_nop=1):
    """walrus in this container rejects >2 sem waits on compute instruction
    structs and >1 on DMA/CTRL structs. Hoist excess waits onto preceding
    same-engine NOPs."""
    import concourse.mybir as mybir
    import bass_rust

    narrow = {"DMACopy", "Drain", "NoOp", "Memset", "TriggeredCopy"}
    cnt = 0
    for bb in nc.m.functions[0].blocks:
        il = bb.instructions
        out = []
        for inst in il:
            cap = 1 if inst.opcode in narrow else max_on_op
            si = inst.sync_info
            waits = list(si.on_wait) if si is not None and si.on_wait else []
            if len(waits) > cap:
                n_extra = len(waits) - cap
                extra, keep = waits[:n_extra], waits[n_extra:]
                for i0 in range(0, len(extra), max_on_nop):
                    chunk = extra[i0 : i0 + max_on_nop]
                    nop = mybir.InstNoOp(name=f"waitnop-{cnt}", ins=[], outs=[])
                    cnt += 1
                    nop.engine = inst.engine
                    nop.sync_info = bass_rust.SyncInfo(on_wait=chunk, on_update=[])
                    out.append(nop)
                inst.sync_info = bass_rust.SyncInfo(
                    on_wait=keep,
                    on_update=list(si.on_update) if si.on_update else [],
                )
            out.append(inst)
        il[:] = out


def _build_program():
    import concourse.bass as bass
    import concourse.tile as tile
    import concourse.mybir as mybir

    _patch_tail_drain()

    f32 = mybir.dt.float32
    bf16 = mybir.dt.bfloat16
    ALU = mybir.AluOpType
    ACT = mybir.ActivationFunctionType

    nc = bass.Bass("TRN2", target_bir_lowering=False, debug=False, num_devices=1)

    x_pm = nc.dram_tensor("x_pm", [S, E], bf16, kind="ExternalInput").ap()
    xT = nc.dram_tensor("xT", [E, S], bf16, kind="ExternalInput").ap()
    wqkvT_d = nc.dram_tensor("wqkvT", [E, 3 * E], bf16, kind="ExternalInput").ap()
    projT_d = nc.dram_tensor("projT", [E, E], bf16, kind="ExternalInput").ap()
    ffw2T_d = nc.dram_tensor("ffw2T", [E, E], bf16, kind="ExternalInput").ap()
    bqkv_d = nc.dram_tensor("bqkv", [1, 3 * E], bf16, kind="ExternalInput").ap()
    bproj_d = nc.dram_tensor("bproj", [1, E], bf16, kind="ExternalInput").ap()
    bff2_d = nc.dram_tensor("bff2", [1, E], bf16, kind="ExternalInput").ap()
    out_d = nc.dram_tensor("out", [S, E], f32, kind="ExternalOutput").ap()

    xT_r = xT.rearrange("(t p) s -> p t s", p=128)  # [128, 8, S]
    wqkv_r = wqkvT_d.rearrange("(t p) o -> p t o", p=128)
    proj_r = projT_d.rearrange("(t p) o -> p t o", p=128)
    ffw2_r = ffw2T_d.rearrange("(t p) o -> p t o", p=128)

    with tile.TileContext(nc) as tc:
        import contextlib

        ctx = contextlib.ExitStack()
        with ctx:
            fixed = ctx.enter_context(tc.tile_pool(name="fixed", bufs=1))
            work = ctx.enter_context(tc.tile_pool(name="work", bufs=2))
            work1 = ctx.enter_context(tc.tile_pool(name="work1", bufs=1))
            stats = ctx.enter_context(tc.tile_pool(name="stats", bufs=2))
            dscr = ctx.enter_context(
                tc.tile_pool(name="dscr", bufs=2, space="DRAM")
            )
            psq = ctx.enter_context(tc.tile_pool(name="psq", bufs=3, space="PSUM"))
            psb = ctx.enter_context(tc.tile_pool(name="psb", bufs=2, space="PSUM"))

            # ---- fixed tensors ----
            wqkv_sb = fixed.tile([128, 8, 3 * E], bf16)
            for t in range(8):
                nc.sync.dma_start(out=wqkv_sb[:, t, :], in_=wqkv_r[:, t, :])
            proj_sb = fixed.tile([128, 8, E], bf16)
            ffw2_sb = fixed.tile([128, 8, E], bf16)
            for t in range(8):
                nc.sync.dma_start(out=proj_sb[:, t, :], in_=proj_r[:, t, :])
                nc.sync.dma_start(out=ffw2_sb[:, t, :], in_=ffw2_r[:, t, :])
            bqkv_sb = fixed.tile([1, 3 * E], bf16)
            nc.sync.dma_start(out=bqkv_sb, in_=bqkv_d)
            bproj_sb = fixed.tile([1, E], bf16)
            nc.sync.dma_start(out=bproj_sb, in_=bproj_d)
            bff2_sb = fixed.tile([1, E], bf16)
            nc.sync.dma_start(out=bff2_sb, in_=bff2_d)
            ones_row = fixed.tile([1, 128], bf16)
            nc.vector.memset(ones_row, 1.0)
            eps_sb = fixed.tile([128, 1], f32)
            nc.vector.memset(eps_sb, EPS)
            # block-diag attn stationaries (zeros persist; diag blocks are
            # rewritten by scatter DMAs each tile) -- manual double buffer
            ablk2 = [
                fixed.tile([128, 16, 8, 16], bf16, name=f"ablk{i}", tag=f"ablk{i}")
                for i in range(2)
            ]
            for a in ablk2:
                nc.vector.memset(a, 0.0)

            inv_n = 1.0 / float(E)

            def layer_norm(s1, s2, rs_out, mrs_out):
                """rsigma and -mu*rsigma from s1=sum(z), s2=sum(z^2)."""
                mu = stats.tile([128, 1], f32, tag="mu")
                nc.vector.tensor_scalar_mul(mu, s1, inv_n)
                mu2 = stats.tile([128, 1], f32, tag="mu2")
                nc.vector.tensor_tensor(mu2, mu, mu, ALU.mult)
                var = stats.tile([128, 1], f32, tag="var")
                nc.vector.scalar_tensor_tensor(
                    var, in0=s2, scalar=inv_n, in1=mu2, op0=ALU.mult, op1=ALU.subtract
                )
                lnv = stats.tile([128, 1], f32, tag="lnv")
                nc.scalar.activation(lnv, var, ACT.Ln, bias=eps_sb)
                nc.scalar.activation(rs_out, lnv, ACT.Exp, scale=-0.5)
                nc.vector.scalar_tensor_tensor(
                    mrs_out, in0=mu, scalar=-1.0, in1=rs_out, op0=ALU.mult, op1=ALU.mult
                )

            def headA(t):
                s0 = t * 128
                xp = work.tile([128, E], bf16, tag="xp", name="xp", bufs=4)
                nc.sync.dma_start(out=xp, in_=x_pm[s0 : s0 + 128, :])
                xf = work.tile([128, 8, 128], bf16, tag="xf", name="xf")
                nc.sync.dma_start(out=xf, in_=xT_r[:, :, s0 : s0 + 128])

                # ---- qkv projection: e-outer for stationary reuse ----
                qkv_sb = work.tile([128, 3 * E], bf16, tag="qkv", name="qkv_sb")
                for wave in range(2):
                    pss = [
                        psq.tile([128, 512], f32, name="psq", tag="psq")
                        for j3 in range(3)
                    ]
                    for e in range(8):
                        for j3 in range(3):
                            j = wave * 3 + j3
                            nc.tensor.matmul(
                                pss[j3],
                                xf[:, e, :],
                                wqkv_sb[:, e, j * 512 : (j + 1) * 512],
                                start=(e == 0),
                                stop=False,
                            )
                    for j3 in range(3):
                        j = wave * 3 + j3
                        nc.tensor.matmul(
                            pss[j3],
                            ones_row,
                            bqkv_sb[:, j * 512 : (j + 1) * 512],
                            start=False,
                            stop=True,
                        )
                    for j3 in range(3):
                        j = wave * 3 + j3
                        nc.scalar.copy(qkv_sb[:, j * 512 : (j + 1) * 512], pss[j3])

                v3 = qkv_sb[:, 2 * E : 3 * E].rearrange("p (g d) -> p g d", g=H)
                v_scr = dscr.tile([128, H, DV], bf16, tag="v_scr", name="v_scr")
                nc.scalar.dma_start(out=v_scr, in_=v3)
                v2 = work.tile([128, H, DV], bf16, tag="v2", name="v2")
                nc.sync.dma_start(
                    out=v2, in_=v_scr.rearrange("(j pl) g d -> pl g j d", pl=8)
                )
                return {"xp": xp, "xf": xf, "qkv_sb": qkv_sb, "s0": s0, "v2": v2}

            def headB(t, st):
                qkv_sb = st["qkv_sb"]
                q3 = qkv_sb[:, 0:E].rearrange("p (h d) -> p h d", h=H)
                k3 = qkv_sb[:, E : 2 * E].rearrange("p (g d) -> p g d", g=H)

                # ---- scoresT[pos, g, h] = sum_d k[g,d] q[h,d] (q pre-scaled) ----
                prod = work1.tile([128, 8, 16, 64], bf16, tag="prod", name="prod")
                scr = work1.tile([128, 6144], bf16, tag="scr", name="scr")
                scoresT = work1.tile([128, H, H], f32, tag="scoresT", name="scoresT")
                for half in range(2):
                    g0 = half * 8
                    kb = (
                        k3[:, g0 : g0 + 8, :]
                        .unsqueeze(2)
                        .broadcast_to([128, 8, 16, 64])
                    )
                    qb = q3.unsqueeze(1).broadcast_to([128, 8, 16, 64])
                    nc.vector.tensor_tensor(prod, kb, qb, ALU.mult)
                    t1 = scr[:, 0:4096].rearrange("p (a h d) -> p a h d", a=8, h=16)
                    nc.vector.tensor_tensor(
                        t1, prod[:, :, :, 0:32], prod[:, :, :, 32:64], ALU.add
                    )
                    t2 = scr[:, 4096:6144].rearrange("p (a h d) -> p a h d", a=8, h=16)
                    nc.vector.tensor_tensor(
                        t2, t1[:, :, :, 0:16], t1[:, :, :, 16:32], ALU.add
                    )
                    t3 = scr[:, 0:1024].rearrange("p (a h d) -> p a h d", a=8, h=16)
                    nc.vector.tensor_tensor(
                        t3, t2[:, :, :, 0:8], t2[:, :, :, 8:16], ALU.add
                    )
                    t4 = scr[:, 1024:1536].rearrange("p (a h d) -> p a h d", a=8, h=16)
                    nc.vector.tensor_tensor(
                        t4, t3[:, :, :, 0:4], t3[:, :, :, 4:8], ALU.add
                    )
                    t5 = scr[:, 1536:1792].rearrange("p (a h d) -> p a h d", a=8, h=16)
                    nc.vector.tensor_tensor(
                        t5, t4[:, :, :, 0:2], t4[:, :, :, 2:4], ALU.add
                    )
                    nc.vector.tensor_tensor(
                        scoresT[:, g0 : g0 + 8, :],
                        t5[:, :, :, 0],
                        t5[:, :, :, 1],
                        ALU.add,
                    )

                # ---- softmax exp (raw; 1/den applied after gather) ----
                p_sb = work.tile([128, H, H], bf16, tag="p_sb", name="p_sb")
                nc.scalar.activation(p_sb, scoresT, ACT.Exp)

                den = stats.tile([128, H], f32, tag="den", name="den")
                nc.vector.tensor_reduce(
                    den,
                    p_sb.rearrange("p g h -> p h g"),
                    axis=mybir.AxisListType.X,
                    op=ALU.add,
                )
                rden = stats.tile([128, H], bf16, tag="rden", name="rden", bufs=4)
                with nc.allow_low_precision(reason="softmax 1/den in bf16"):
                    nc.vector.reciprocal(rden, den)
                p_n = work1.tile([128, H, H], bf16, tag="p_n", name="p_n")
                rb = rden.unsqueeze(1).broadcast_to([128, H, H])
                nc.vector.tensor_tensor(p_n, p_sb, rb, ALU.mult)

                # ---- scatter normalized attn to the block-diag layout ----
                p_scr = dscr.tile([128, H, H], bf16, tag="p_scr", name="p_scr")
                nc.sync.dma_start(out=p_scr, in_=p_n)
                ablk = ablk2[t % 2]
                psr = p_scr.rearrange("(j pl) g h -> g j pl h", pl=8)
                for pl in range(8):
                    eng = (nc.sync, nc.scalar, nc.gpsimd, nc.gpsimd)[pl % 4]
                    eng.dma_start(
                        out=ablk[pl * 16 : (pl + 1) * 16, :, pl, :],
                        in_=psr[:, :, pl, :],
                    )
                st["ablk"] = ablk
                return st

            def tailA(st):
                xp, v2, ablk, s0 = st["xp"], st["v2"], st["ablk"], st["s0"]
                # ---- attn @ v on TensorE: 16 block-diag matmuls ----
                avps = [
                    psb.tile([128, 512], f32, name="psb", tag="psb") for i in range(2)
                ]
                for jg in range(16):
                    ps = avps[jg // 8]
                    col = (jg % 8) * 64
                    nc.tensor.matmul(
                        ps[:, col : col + 64],
                        ablk[:, jg, :, :].rearrange("p a b -> p (a b)"),
                        v2[:, jg, :],
                        start=True,
                        stop=True,
                    )
                avout = work.tile([128, H, DV], bf16, tag="avout", name="avout")
                nc.scalar.copy(
                    avout[:, 0:8, :].rearrange("p a b -> p (a b)"), avps[0]
                )
                nc.scalar.copy(
                    avout[:, 8:16, :].rearrange("p a b -> p (a b)"), avps[1]
                )

                # ---- gather back to position-major [pos, (h,d)] ----
                av_scr = dscr.tile(
                    [128, H, DV], bf16, tag="av_scr", name="av_scr", bufs=3
                )
                nc.scalar.dma_start(out=av_scr, in_=avout)
                st["av_scr"] = av_scr

            def tailA2(st):
                xp, s0, av_scr = st["xp"], st["s0"], st["av_scr"]
                attn_bf = work.tile([128, H, DV], bf16, tag="attn_bf", name="attn_bf")
                nc.sync.dma_start(
                    out=attn_bf, in_=av_scr.rearrange("(pl h) j d -> j pl h d", pl=8)
                )

                # ---- feature-major via PE transpose ----
                attn_flat = attn_bf.rearrange("p a b -> p (a b)")
                attn_fm = work.tile([128, 8, 128], bf16, tag="attn_fm", name="attn_fm")
                for e in range(8):
                    pt = pst.tile([128, 128], bf16, name="pst", tag="pst")
                    nc.tensor.transpose(
                        pt, attn_flat[:, e * 128 : (e + 1) * 128], ident
                    )
                    nc.scalar.copy(attn_fm[:, e, :], pt)

                # ---- proj + residual ----
                z1 = work.tile([128, E], f32, tag="z1", name="z1")
                pps = [
                    psb.tile([128, 512], f32, name="psb", tag="psb") for i in range(2)
                ]
                for e in range(8):
                    for i in range(2):
                        nc.tensor.matmul(
                            pps[i],
                            attn_fm[:, e, :],
                            proj_sb[:, e, i * 512 : (i + 1) * 512],
                            start=(e == 0),
                            stop=False,
                        )
                for i in range(2):
                    nc.tensor.matmul(
                        pps[i],
                        ones_row,
                        bproj_sb[:, i * 512 : (i + 1) * 512],
                        start=False,
                        stop=True,
                    )
                s1a = stats.tile([128, 1], f32, tag="s1a", name="s1a")
                s1b = stats.tile([128, 1], f32, tag="s1b", name="s1b")
                for i in range(2):
                    nc.vector.scalar_tensor_tensor(
                        z1[:, i * 512 : (i + 1) * 512],
                        in0=pps[i],
                        scalar=1.0,
                        in1=xp[:, i * 512 : (i + 1) * 512],
                        op0=ALU.mult,
                        op1=ALU.add,
                        accum_out=s1a if i == 0 else s1b,
                    )
                nc.vector.tensor_tensor(s1b, s1a, s1b, ALU.add)
                s2a = stats.tile([128, 1], f32, tag="s2a", name="s2a")
                s2b = stats.tile([128, 1], f32, tag="s2b", name="s2b")
                zsq = work1.tile([128, 512], bf16, tag="zsq", name="zsq")
                for i in range(2):
                    nc.scalar.activation(
                        zsq,
                        z1[:, i * 512 : (i + 1) * 512],
                        ACT.Square,
                        accum_out=s2a if i == 0 else s2b,
                    )
                nc.vector.tensor_tensor(s2a, s2a, s2b, ALU.add)

                # ---- LN1 (g,b folded into ff weights) ----
                rs1 = stats.tile([128, 1], f32, tag="rs1", name="rs1")
                mrs1 = stats.tile([128, 1], f32, tag="mrs1", name="mrs1")
                layer_norm(s1b, s2a, rs1, mrs1)
                ln1_bf = work.tile(
                    [128, E], bf16, tag="ln1_bf", name="ln1_bf", bufs=3
                )
                nc.scalar.activation(ln1_bf, z1, ACT.Identity, bias=mrs1, scale=rs1)
                st["ln1_bf"] = ln1_bf

            def tailB(st):
                xp, s0, ln1_bf = st["xp"], st["s0"], st["ln1_bf"]
                ln1_fm = work.tile([128, 8, 128], bf16, tag="ln1_fm", name="ln1_fm")
                for e in range(8):
                    pt = pst.tile([128, 128], bf16, name="pst", tag="pst")
                    nc.tensor.transpose(
                        pt, ln1_bf[:, e * 128 : (e + 1) * 128], ident
                    )
                    nc.scalar.copy(ln1_fm[:, e, :], pt)

                # ---- ff + gelu + residual ----
                z2 = work.tile([128, E], f32, tag="z2", name="z2")
                gl = work1.tile([128, E], bf16, tag="gl", name="gl")
                fps = [
                    psb.tile([128, 512], f32, name="psb", tag="psb") for i in range(2)
                ]
                for e in range(8):
                    for i in range(2):
                        nc.tensor.matmul(
                            fps[i],
                            ln1_fm[:, e, :],
                            ffw2_sb[:, e, i * 512 : (i + 1) * 512],
                            start=(e == 0),
                            stop=False,
                        )
                for i in range(2):
                    nc.tensor.matmul(
                        fps[i],
                        ones_row,
                        bff2_sb[:, i * 512 : (i + 1) * 512],
                        start=False,
                        stop=True,
                    )
                t1a = stats.tile([128, 1], f32, tag="t1a", name="t1a")
                t1b = stats.tile([128, 1], f32, tag="t1b", name="t1b")
                for i in range(2):
                    nc.scalar.activation(
                        gl[:, i * 512 : (i + 1) * 512], fps[i], ACT.Gelu
                    )
                    nc.vector.scalar_tensor_tensor(
                        z2[:, i * 512 : (i + 1) * 512],
                        in0=gl[:, i * 512 : (i + 1) * 512],
                        scalar=1.0,
                        in1=xp[:, i * 512 : (i + 1) * 512],
                        op0=ALU.mult,
                        op1=ALU.add,
                        accum_out=t1a if i == 0 else t1b,
                    )
                nc.vector.tensor_tensor(t1b, t1a, t1b, ALU.add)
                t2a = stats.tile([128, 1], f32, tag="t2a", name="t2a")
                t2b = stats.tile([128, 1], f32, tag="t2b", name="t2b")
                zsq2 = work1.tile([128, 512], bf16, tag="zsq", name="zsq")
                for i in range(2):
                    nc.scalar.activation(
                        zsq2,
                        z2[:, i * 512 : (i + 1) * 512],
                        ACT.Square,
                        accum_out=t2a if i == 0 else t2b,
                    )
                nc.vector.tensor_tensor(t2a, t2a, t2b, ALU.add)

                # ---- LN2 core (affine applied on host) ----
                rs2 = stats.tile([128, 1], f32, tag="rs2", name="rs2")
                mrs2 = stats.tile([128, 1], f32, tag="mrs2", name="mrs2")
                layer_norm(t1b, t2a, rs2, mrs2)
                out_t = work.tile([128, E], f32, tag="out_t", name="out_t")
                nc.scalar.activation(out_t, z2, ACT.Identity, bias=mrs2, scale=rs2)
                nc.sync.dma_start(out=out_d[s0 : s0 + 128, :], in_=out_t)

            # 3-stage software pipeline: per iteration emit
            # [headA(t) | tailA(t-1) | tailB(t-2) | headB(t)] so long-wait ops
            # sit late in each engine's in-order stream and the LN1 chain of
            # tile t-1 overlaps tile t-2's ff matmuls
            sp1 = sp2 = sp3 = None
            for t in range(NT):
                cur = headA(t)
                if sp1 is not None:
                    tailA(sp1)
                if sp2 is not None:
                    tailA2(sp2)
                if sp3 is not None:
                    tailB(sp3)
                cur = headB(t, cur)
                sp1, sp2, sp3 = cur, sp1, sp2
            tailA(sp1)
            tailA2(sp2)
            tailB(sp3)
            tailA2(sp1)
            tailB(sp2)
            tailB(sp1)

    _split_excess_waits(nc)
    return nc


def _host_prep(inputs):
    x = np.asarray(inputs["x"], np.float32)
    qk_w = np.asarray(inputs["qk_w"], np.float32)
    qk_b = np.asarray(inputs["qk_b"], np.float32)
    v_w = np.asarray(inputs["v_w"], np.float32)
    v_b = np.asarray(inputs["v_b"], np.float32)
    proj_w = np.asarray(inputs["proj_w"], np.float32)
    proj_b = np.asarray(inputs["proj_b"], np.float32)
    ff_w = np.asarray(inputs["ff_w"], np.float32)
    ff_b = np.asarray(inputs["ff_b"], np.float32)
    ln_g = np.asarray(inputs["ln_g"], np.float32)
    ln_b = np.asarray(inputs["ln_b"], np.float32)

    scale = 1.0 / np.sqrt(DQ).astype(np.float32)
    Wq = qk_w[:E] * scale
    bq = qk_b[:E] * scale
    Wk = qk_w[E:]
    bk = qk_b[E:]

    wqkvT = np.ascontiguousarray(
        np.concatenate([Wq, Wk, v_w], 0).T.astype(BF)
    )  # [E, 3E]
    bqkv = np.concatenate([bq, bk, v_b])[None, :].astype(BF)  # [1, 3E]
    projT = np.ascontiguousarray(proj_w.T.astype(BF))  # [E, E]
    bproj = proj_b[None, :].astype(BF)
    ffw2T = np.ascontiguousarray((ff_w * ln_g[None, :]).T.astype(BF))
    bff2 = (ff_b + ff_w @ ln_b)[None, :].astype(BF)

    shared = {
        "wqkvT": wqkvT,
        "bqkv": bqkv,
        "projT": projT,
        "bproj": bproj,
        "ffw2T": ffw2T,
        "bff2": bff2,
    }
    in_maps = []
    for b in range(B):
        xb = np.ascontiguousarray(x[b])  # [S, E] f32
        xTb = np.ascontiguousarray(xb.T.astype(BF))  # [E, S] bf16
        m = {"x_pm": xb.astype(BF), "xT": xTb}
        m.update(shared)
        in_maps.append(m)
    return in_maps


def kernel(**inputs) -> np.ndarray:
    from concourse.bass_utils import run_bass_kernel_spmd

    if "nc" not in _CACHE:
        _CACHE["nc"] = _build_program()
    nc = _CACHE["nc"]

    in_maps = _host_prep(inputs)
    res = run_bass_kernel_spmd(nc, in_maps, core_ids=list(range(B)))
    out = np.stack([res.results[b]["out"] for b in range(B)], 0).astype(np.float32)

    # LN2 affine applied host-side (kernel returns the normalized core)
    ln_g = np.asarray(inputs["ln_g"], np.float32)
    ln_b = np.asarray(inputs["ln_b"], np.float32)
    if not (np.all(ln_g == 1.0) and np.all(ln_b == 0.0)):
        out = out * ln_g[None, None, :] + ln_b[None, None, :]
    return out


if __name__ == "__main__":
    rng = np.random.default_rng(0)
    ins = {
        "x": rng.standard_normal((B, S, E), np.float32),
        "qk_w": rng.standard_normal((2 * E, E), np.float32) * 0.03,
        "qk_b": rng.standard_normal((2 * E,), np.float32) * 0.03,
        "v_w": rng.standard_normal((E, E), np.float32) * 0.03,
        "v_b": rng.standard_normal((E,), np.float32) * 0.03,
        "proj_w": rng.standard_normal((E, E), np.float32) * 0.03,
        "proj_b": rng.standard_normal((E,), np.float32) * 0.03,
        "ff_w": rng.standard_normal((E, E), np.float32) * 0.03,
        "ff_b": rng.standard_normal((E,), np.float32) * 0.03,
        "ln_g": np.ones((E,), np.float32),
        "ln_b": np.zeros((E,), np.float32),
    }
    o = kernel(**ins)
    print("ran", o.shape, o.dtype)
